# revision 13
# baseline (speedup 1.0000x reference)
"""GCN encoder (3-layer GCNConv + LayerNorm + ReLU + residual) on 8 TRN2
NeuronCores via Bass/Tile.

Sharding: nodes are partitioned across cores (graph parallel), degree-sorted
so each 128-node window has near-uniform in-degree. Per layer each core
computes xw = dinv*(h @ W^T) for its shard, AllGathers the bf16 table, then
aggregates in two paths:
  - PE path (28 highest-degree windows): out^T = table^T @ A, where A is the
    fp8-quantized weighted adjacency slab [src, dest] streamed from DRAM,
    accumulated over 392 source tiles in PSUM (7 banks x 4 windows).
  - Gather path (21 remaining windows): per-k indirect DMA row gathers +
    DVE multiply/segment-reduce (padded CSR).
LayerNorm/ReLU/residual tail is shared.
"""

import numpy as np
import ml_dtypes

import concourse.bacc as bacc
import concourse.bass as bass
import concourse.mybir as mybir
from concourse.tile import TileContext
from concourse.bass_utils import run_bass_kernel_spmd

F32 = mybir.dt.float32
BF16 = mybir.dt.bfloat16
FP8 = mybir.dt.float8e4
I32 = mybir.dt.int32
AX = mybir.AxisListType
ALU = mybir.AluOpType
ACTF = mybir.ActivationFunctionType

FP8_NP = mybir.dt.np(FP8)
ASCALE = 128.0          # fp8 pre-scale: A in [0,1] -> [0,128], e4m3 max 240


# ----------------------------------------------------------------------------
# Host-side structure packing
# ----------------------------------------------------------------------------

def build_structure(edge_index, edge_weight, N, C, W, XPE):
    """Partition nodes across C cores, degree-sort each core's dests, build
    fp8 adjacency slabs for the XPE highest-degree windows and padded-CSR
    images for the rest."""
    NPC = N // C
    NP = W * 128
    src = edge_index[0].astype(np.int64)
    dst = edge_index[1].astype(np.int64)
    E = src.shape[0]

    loop = np.arange(N, dtype=np.int64)
    src2 = np.concatenate([src, loop])
    dst2 = np.concatenate([dst, loop])
    w2 = np.concatenate([np.asarray(edge_weight, np.float64), np.ones(N)])

    deg = np.bincount(dst2, weights=w2, minlength=N)
    dinv = np.where(deg > 0, 1.0 / np.sqrt(np.maximum(deg, 1e-12)), 0.0)

    cnt = np.bincount(dst2, minlength=N)       # slot count (incl self loop)

    rank = np.empty(N, dtype=np.int64)
    for c in range(C):
        lo, hi = c * NPC, (c + 1) * NPC
        order = np.argsort(-cnt[lo:hi], kind="stable")
        rank[lo + order] = np.arange(NPC)
    node_pos = (np.arange(N) // NPC) * NP + rank   # global table row

    owner = dst2 // NPC
    wnorm = w2 * dinv[dst2]                    # full coeff sans dinv[src]

    NPE = XPE * 128                            # PE-path dests per core
    cores = []
    KG = np.zeros((C, W - XPE), dtype=np.int64)
    for c in range(C):
        sel = owner == c
        e_dst = dst2[sel]
        dloc = rank[e_dst]                     # local dest rank [0, NPC)
        spos = node_pos[src2[sel]]             # global table row of source
        wv = wnorm[sel]

        pe = dloc < NPE
        # --- PE slab: A[src_row, dloc] += wnorm (dup edges accumulate) ---
        A = np.zeros((NP * C, NPE), dtype=np.float32)
        np.add.at(A, (spos[pe], dloc[pe]), wv[pe])
        A8 = (A * ASCALE).astype(FP8_NP)

        # --- gather-path padded CSR for windows XPE..W-1 ---
        gm = ~pe
        gd = dloc[gm] - NPE
        gs = spos[gm]
        gw = wv[gm]
        o = np.argsort(gd, kind="stable")
        gd, gs, gw = gd[o], gs[o], gw[o]
        ng = NP - NPE
        cntg = np.bincount(gd, minlength=ng)
        starts = np.zeros(ng, dtype=np.int64)
        starts[1:] = np.cumsum(cntg)[:-1]
        vw = np.arange(ng) // 128
        for w in range(W - XPE):
            m = vw == w
            KG[c, w] = cntg[m].max() if m.any() else 0
        cores.append(dict(gd=gd, gs=gs, gw=gw, cntg=cntg, starts=starts,
                          A8=A8))

    KG = KG.max(axis=0)
    return dict(NPC=NPC, NP=NP, C=C, W=W, XPE=XPE, NPE=NPE, KG=KG,
                cores=cores, rank=rank, dinv=dinv)


def _pad_block(vals, starts, lens, K, fill):
    col = np.arange(K)[None, :]
    mask = col < lens[:, None]
    sp = np.where(mask, starts[:, None] + col, 0)
    return np.where(mask, vals[sp], fill)


def pack_core(st, c):
    """int32 index image + f32 coeff image for core c's gather windows."""
    W, XPE, KG = st["W"], st["XPE"], st["KG"]
    d = st["cores"][c]
    idx_cols, w_cols = [], []
    for w in range(W - XPE):
        vs = slice(w * 128, (w + 1) * 128)
        stt = d["starts"][vs]
        ln = d["cntg"][vs]
        K = int(KG[w])
        if K == 0:
            continue
        pi = _pad_block(d["gs"], stt, ln, K, 0)
        pw = _pad_block(d["gw"], stt, ln, K, 0.0)
        idx_cols.append(pi.astype(np.int32))
        w_cols.append(pw.astype(ml_dtypes.bfloat16))
    return np.concatenate(idx_cols, 1), np.concatenate(w_cols, 1)


# ----------------------------------------------------------------------------
# Bass program
# ----------------------------------------------------------------------------

def build_program(st, L, D=128):
    W, XPE = st["W"], st["XPE"]
    NP, NPE, C = st["NP"], st["NPE"], st["C"]
    KG = st["KG"]
    WG = W - XPE                        # gather windows
    KT = [int(KG[w]) for w in range(WG)]
    KCOLS = int(sum(KT))
    NT = NP * C                         # table rows
    NS = NT // 128                      # source tiles
    NB = XPE // 4                       # PSUM banks for PE path (4 win each)

    nc = bacc.Bacc("TRN2", target_bir_lowering=False, debug=True)

    x_in = nc.dram_tensor("x_shard", [NP, D], F32, kind="ExternalInput")
    idx_in = nc.dram_tensor("idx_img", [128, KCOLS], I32, kind="ExternalInput")
    w_in = nc.dram_tensor("w_img", [128, KCOLS], BF16, kind="ExternalInput")
    a_in = nc.dram_tensor("a_img", [NT, NPE], FP8, kind="ExternalInput")
    dinv_in = nc.dram_tensor("dinv_img", [128, W], F32, kind="ExternalInput")
    wst_in = nc.dram_tensor("wst", [L, D, D], F32, kind="ExternalInput")
    bias_in = nc.dram_tensor("bias_b", [L, D, D], F32, kind="ExternalInput")
    gam_in = nc.dram_tensor("gamma_b", [L, D, D], F32, kind="ExternalInput")
    bet_in = nc.dram_tensor("beta_b", [L, D, D], F32, kind="ExternalInput")
    id_in = nc.dram_tensor("ident", [D, D], F32, kind="ExternalInput")
    out_t = nc.dram_tensor("out_shard", [NP, D], F32, kind="ExternalOutput")

    with TileContext(nc) as tc:
        with (
            tc.tile_pool(name="persist", bufs=1) as pp,
            tc.tile_pool(name="gath", bufs=3) as gp,
            tc.tile_pool(name="astage", bufs=4) as ap_,
            tc.tile_pool(name="work", bufs=3) as wk,
            tc.tile_pool(name="tiny", bufs=4) as tn,
            tc.tile_pool(name="psacc", bufs=1, space="PSUM") as pa,
            tc.tile_pool(name="dram", bufs=1, space="DRAM") as dr,
        ):
            # ---- persistent SBUF state ----
            h = pp.tile([128, W, D], F32, tag="h")
            idx = pp.tile([128, KCOLS], I32, tag="idx")
            wn = pp.tile([128, KCOLS], BF16, tag="wn")
            wst = pp.tile([128, L * D], F32, tag="wst")
            biasb = pp.tile([128, L * D], F32, tag="biasb")
            gamb = pp.tile([128, L * D], F32, tag="gamb")
            betb = pp.tile([128, L * D], F32, tag="betb")
            ident = pp.tile([128, D], F32, tag="ident")
            dinv = pp.tile([128, W], F32, tag="dinv")
            tabsb = pp.tile([128, NS, D], BF16, tag="tabsb")

            nc.sync.dma_start(out=h[:, :, :],
                              in_=x_in[:].rearrange("(w p) f -> p w f", p=128))
            nc.sync.dma_start(out=idx[:, :], in_=idx_in[:, :])
            nc.sync.dma_start(out=wn[:, :], in_=w_in[:, :])
            nc.sync.dma_start(out=dinv[:, :], in_=dinv_in[:, :])
            for l in range(L):
                for dst_t, src_t in ((wst, wst_in), (biasb, bias_in),
                                     (gamb, gam_in), (betb, bet_in)):
                    nc.sync.dma_start(out=dst_t[:, l * D:(l + 1) * D],
                                      in_=src_t[l, :, :])
            nc.sync.dma_start(out=ident[:, :], in_=id_in[:, :])

            # ---- per-layer DRAM tables (bf16, double buffered) ----
            tables = [dr.tile([NT, D], BF16, name=f"table{i}", tag=f"table{i}")
                      for i in range(2)]
            xw_own = [dr.tile([NP, D], BF16, name=f"xwown{i}", tag=f"xwown{i}")
                      for i in range(2)]

            for li in range(L):
                tab = tables[li % 2]
                own = xw_own[li % 2]
                wst_l = wst[:, li * D:(li + 1) * D]
                bias_l = biasb[:, li * D:(li + 1) * D]
                gam_l = gamb[:, li * D:(li + 1) * D]
                bet_l = betb[:, li * D:(li + 1) * D]

                accs = [pa.tile([128, 4 * D], F32, name=f"acc{b}",
                                tag=f"acc{b}") for b in range(NB)]

                # -- own table shard: T = dinv * (h @ Ws^T), bf16 --
                # gather windows first (their h is updated earliest); build
                # transposes/matmuls pipeline 4-deep through acc bank slices
                border = list(range(XPE, W)) + list(range(XPE))
                for i, w in enumerate(border):
                    q = i % 4
                    hT = accs[0][:, q * D:(q + 1) * D]
                    nc.tensor.transpose(hT, h[:, w, :], ident[:, :])
                    hTs = wk.tile([128, D], F32, tag="hTs")
                    nc.scalar.activation(hTs[:, :], hT, ACTF.Copy)
                    mm = accs[1][:, q * D:(q + 1) * D]
                    nc.tensor.matmul(mm, hTs[:, :], wst_l)
                    xw = wk.tile([128, D], BF16, tag="xw")
                    nc.scalar.activation(xw[:, :], mm, ACTF.Copy,
                                         scale=dinv[:, w:w + 1])
                    nc.sync.dma_start(out=own[w * 128:(w + 1) * 128, :],
                                      in_=xw[:, :])
                nc.gpsimd.collective_compute(
                    "AllGather", ALU.bypass,
                    replica_groups=[list(range(C))],
                    ins=[own[:].opt()], outs=[tab[:].opt()])
                nc.sync.dma_start(
                    out=tabsb[:, :, :],
                    in_=tab[:].rearrange("(s p) f -> p s f", p=128))

                # ================= gather path: windows XPE.. ============
                off_k = 0
                for wg in range(WG):
                    kt = KT[wg]
                    w = XPE + wg
                    g = gp.tile([128, kt, D], BF16, tag="g")
                    for k in range(kt):
                        nc.gpsimd.indirect_dma_start(
                            out=g[:, k, :], out_offset=None,
                            in_=tab[:, :],
                            in_offset=bass.IndirectOffsetOnAxis(
                                ap=idx[:, off_k + k:off_k + k + 1], axis=0))
                    nw = wn[:, off_k:off_k + kt].unsqueeze(2)
                    nc.vector.tensor_tensor(
                        g[:, :, :], g[:, :, :],
                        nw.broadcast_to([128, kt, D]), ALU.mult)
                    agg = wk.tile([128, D], F32, tag="agg")
                    nc.vector.tensor_reduce(
                        agg[:, :], g[:, :, :].transpose([0, 2, 1]),
                        AX.X, ALU.add)
                    x0 = wk.tile([128, D], F32, tag="x0")
                    nc.vector.tensor_add(x0[:, :], agg[:, :], bias_l)
                    _tail(nc, tn, wk, h, x0, w, li, L, gam_l, bet_l)
                    off_k += kt

                # ================= PE path: windows 0..XPE-1 =============
                for s in range(NS):
                    asl = ap_.tile([128, NPE], FP8, tag="asl")
                    nc.sync.dma_start(out=asl[:, :],
                                      in_=a_in[s * 128:(s + 1) * 128, :])
                    for b in range(NB):
                        nc.tensor.matmul(
                            accs[b][:, :], tabsb[:, s, :],
                            asl[:, b * 4 * D:(b + 1) * 4 * D],
                            start=(s == 0), stop=(s == NS - 1),
                            skip_group_check=True)

                # drain accumulators: transpose back to node-major + tail
                for b in range(NB):
                    accsb = wk.tile([128, 4 * D], F32, tag="accsb")
                    nc.scalar.activation(accsb[:, :], accs[b][:, :],
                                         ACTF.Copy, scale=1.0 / ASCALE)
                    for q in range(4):
                        w = b * 4 + q
                        tps = accs[b][:, q * D:(q + 1) * D]
                        nc.tensor.transpose(tps,
                                            accsb[:, q * D:(q + 1) * D],
                                            ident[:, :])
                        x0 = wk.tile([128, D], F32, tag="x0")
                        nc.vector.tensor_add(x0[:, :], tps, bias_l)
                        _tail(nc, tn, wk, h, x0, w, li, L, gam_l, bet_l)

            nc.sync.dma_start(out=out_t[:].rearrange("(w p) f -> p w f", p=128),
                              in_=h[:, :, :])

    nc.compile()
    return nc


def _tail(nc, tn, wk, h, x0, w, li, L, gam_l, bet_l):
    """LayerNorm + (ReLU) + residual into h[:, w, :]."""
    D = 128
    sx = tn.tile([128, 1], F32, tag="sx")
    nc.vector.tensor_reduce(sx[:, :], x0[:, :], AX.X, ALU.add)
    sq = tn.tile([128, 1], F32, tag="sq")
    sqs = wk.tile([128, D], F32, tag="sqs")
    nc.scalar.activation(sqs[:, :], x0[:, :], ACTF.Square, accum_out=sq[:, :])
    mu = tn.tile([128, 1], F32, tag="mu")
    nc.vector.tensor_scalar_mul(mu[:, :], sx[:, :], 1.0 / D)
    ms = tn.tile([128, 1], F32, tag="ms")
    nc.vector.tensor_scalar(ms[:, :], sq[:, :], 1.0 / D, 1e-5, ALU.mult,
                            ALU.add)
    mu2 = tn.tile([128, 1], F32, tag="mu2")
    nc.vector.tensor_mul(mu2[:, :], mu[:, :], mu[:, :])
    var = tn.tile([128, 1], F32, tag="var")
    nc.vector.tensor_sub(var[:, :], ms[:, :], mu2[:, :])
    rv = tn.tile([128, 1], F32, tag="rv")
    nc.vector.reciprocal(rv[:, :], var[:, :])
    rstd = tn.tile([128, 1], F32, tag="rstd")
    nc.scalar.sqrt(rstd[:, :], rv[:, :])
    nmr = tn.tile([128, 1], F32, tag="nmr")
    nc.vector.tensor_mul(nmr[:, :], mu[:, :], rstd[:, :])
    t = wk.tile([128, D], F32, tag="t")
    nc.vector.tensor_scalar(t[:, :], x0[:, :], rstd[:, :], nmr[:, :],
                            ALU.mult, ALU.subtract)
    nc.vector.tensor_mul(t[:, :], t[:, :], gam_l)
    nc.vector.tensor_add(t[:, :], t[:, :], bet_l)
    if li < L - 1:
        nc.scalar.activation(t[:, :], t[:, :], ACTF.Relu)
    nc.vector.tensor_add(h[:, w, :], t[:, :], h[:, w, :])


# ----------------------------------------------------------------------------
# Full kernel entry
# ----------------------------------------------------------------------------

def _kernel_impl(x, edge_index, edge_weight, Ws, bs, gammas, betas,
                 C=8, W=49, XPE=28, trace=False):
    N, D = x.shape
    L = Ws.shape[0]
    st = build_structure(edge_index, edge_weight, N, C, W, XPE)
    NP, NPC = st["NP"], st["NPC"]

    ident = np.eye(D, dtype=np.float32)
    wst = np.ascontiguousarray(np.transpose(np.asarray(Ws), (0, 2, 1)))
    bias_b = np.ascontiguousarray(np.broadcast_to(
        np.asarray(bs)[:, None, :], (L, D, D))).astype(np.float32)
    gam_b = np.ascontiguousarray(np.broadcast_to(
        np.asarray(gammas)[:, None, :], (L, D, D))).astype(np.float32)
    bet_b = np.ascontiguousarray(np.broadcast_to(
        np.asarray(betas)[:, None, :], (L, D, D))).astype(np.float32)

    in_maps = []
    for c in range(C):
        idx_img, w_img = pack_core(st, c)
        xs = np.zeros((NP, D), dtype=np.float32)
        lo = c * NPC
        xs[st["rank"][lo:lo + NPC]] = np.asarray(x[lo:lo + NPC],
                                                 dtype=np.float32)
        dv = np.ones(NP, dtype=np.float32)
        dv[st["rank"][lo:lo + NPC]] = st["dinv"][lo:lo + NPC]
        dinv_img = np.ascontiguousarray(
            dv.reshape(W, 128).T).astype(np.float32)
        in_maps.append(dict(x_shard=xs, idx_img=idx_img, w_img=w_img,
                            a_img=st["cores"][c]["A8"], dinv_img=dinv_img,
                            wst=wst, bias_b=bias_b, gamma_b=gam_b,
                            beta_b=bet_b, ident=ident))

    nc = build_program(st, L, D)
    res = run_bass_kernel_spmd(nc, in_maps, list(range(C)), trace=trace)

    out = np.empty((N, D), dtype=np.float32)
    for c in range(C):
        lo = c * NPC
        sh = res.results[c]["out_shard"]
        out[lo:lo + NPC] = sh[st["rank"][lo:lo + NPC]]
    return out, res


def kernel(x, edge_index, edge_weight, Ws, bs, gammas, betas):
    return _kernel_impl(np.asarray(x), np.asarray(edge_index),
                        np.asarray(edge_weight), np.asarray(Ws),
                        np.asarray(bs), np.asarray(gammas),
                        np.asarray(betas))[0]
